# revision 19
# baseline (speedup 1.0000x reference)
"""Multi-head attention kernel for Trainium2, sharded over 8 NeuronCores.

Problem: B=4, S=2048, D=768, H=12 heads of dim 64.
  qkv = x @ w_qkv + b_qkv ; attention per head ; out = concat @ w_proj + b_proj

Sharding (batch x head-group): core c handles batch b = c//2 and head group
g = c%2 (6 heads, 384 qkv columns / w_proj rows).  Each core computes its
partial projection output; the host sums the two partials per batch and adds
the bias.

Algebraic simplifications (exact up to float rounding):
  - k-bias drops out of softmax entirely: (q+bq)@(k+bk)^T differs from
    (q+bq)@k^T by a per-query constant, which softmax cancels.
  - v-bias commutes with the normalized attention average, so it is folded
    into the host-side output bias: b_eff = b_proj + b_qkv[v] @ w_proj.

Device dataflow per core (fp16 matmul operands, fp32 PSUM accumulation):
  xT   = transpose(x)  (PE transpose, 4 column-group tiles)
  QT/KT = w^T @ x^T (+bq on Q)              [384, S] fp16
  V    = x @ wv, + ones column              [S, 6, 65] fp16
  per head h: scoresT = k_h @ q_h^T ; attnT = exp(scoresT/8)  (ScalarE)
              o|rowsum = attnT^T @ [v|1]  accumulated over key chunks
              attn_out = o * (1/rowsum)
  AOT  = transpose(attn_out) ; y = AOT^T @ w_proj_slice  [S, 768] fp32

The emission is software-pipelined around ScalarE (exp = 96 x [128,2048]
activations, the per-core bottleneck): each head iteration carries the
previous head's attn@v second half, this head's first half (two query
chunks merged into one PSUM slot + one strided copy), V chunks (head 0),
deferred QT/KT column groups, and AOT transposes for finished head pairs.
PSUM is two rotating [128,2048] fp32 slots; attn@v accumulates across the
two 8-key-chunk halves via small SBUF accumulators.  DMA load is spread
over the sync/scalar/gpsimd queues to keep the x stream on the critical
path.
"""

import os
import sys
from contextlib import ExitStack

import numpy as np

for _p in ("/opt/trn_rl_repo",):
    if os.path.isdir(_p) and _p not in sys.path:
        sys.path.insert(0, _p)

import concourse.bass as bass  # noqa: E402
import concourse.tile as tile  # noqa: E402
from concourse import bacc, mybir  # noqa: E402
from concourse.masks import make_identity  # noqa: E402

B, S, D, H = 4, 2048, 768, 12
HD = 64  # head dim
HPC = 6  # heads per core
GC = HPC * HD  # 384 qkv columns per core
P = 128
N_CORES = 8
SC = S // P  # 16 sequence chunks
KC = D // P  # 6 contraction chunks over D
MC = GC // P  # 3 column chunks per group
NT = 512  # matmul moving-dim tile
NSG = S // NT  # 4 sequence groups of 512

F32 = mybir.dt.float32
F16 = mybir.dt.float16

ATTNT_BUFS = 17  # [128, S] fp16 exp-output tiles in flight
ACC_BUFS = 10  # [128, 2, 65] fp32 attn@v pair accumulators in flight


def _build_bass():
    nc = bacc.Bacc("TRN2", target_bir_lowering=False, debug=False)
    x = nc.dram_tensor("x", (S, D), F32, kind="ExternalInput").ap()
    wq = nc.dram_tensor("wq", (D, GC), F32, kind="ExternalInput").ap()
    wk = nc.dram_tensor("wk", (D, GC), F32, kind="ExternalInput").ap()
    wv = nc.dram_tensor("wv", (D, GC), F32, kind="ExternalInput").ap()
    bq = nc.dram_tensor("bq", (GC,), F32, kind="ExternalInput").ap()
    wp = nc.dram_tensor("wp", (GC, D), F32, kind="ExternalInput").ap()
    y = nc.dram_tensor("y", (S, D), F32, kind="ExternalOutput").ap()
    with tile.TileContext(nc) as tc:
        _mha_kernel(tc, y, x, wq, wk, wv, bq, wp)
    nc.finalize()
    return nc


def _mha_kernel(tc, y, x, wq, wk, wv, bq, wp):
    nc = tc.nc
    with ExitStack() as ctx:
        # Two 4-bank [128, 2048] fp32 PSUM slots shared by every psum user.
        psum = ctx.enter_context(tc.tile_pool(name="psum", bufs=2, space="PSUM"))
        persist = ctx.enter_context(tc.tile_pool(name="persist", bufs=1))
        small = ctx.enter_context(tc.tile_pool(name="small", bufs=6))

        idf = persist.tile([P, P], F32, name="idf")
        make_identity(nc, idf)
        idh = persist.tile([P, P], F16, name="idh")
        make_identity(nc, idh)

        QT = persist.tile([P, MC, S], F16, name="QT")
        KT = persist.tile([P, MC, S], F16, name="KT")
        V = persist.tile([P, SC, HPC, HD + 1], F16, name="V")
        AO = persist.tile([P, SC, GC], F16, name="AO")
        AOT = persist.tile([P, MC, S], F16, name="AOT")
        bq_sb = persist.tile([P, MC], F32, name="bq_sb")
        nc.gpsimd.dma_start(out=bq_sb, in_=bq.rearrange("(mc p) -> p mc", p=P))
        nc.vector.memset(V[:, :, :, HD : HD + 1], 1.0)

        pa = ctx.enter_context(tc.tile_pool(name="pa", bufs=1))
        # x^T in four 512-query groups so the qkv matmuls can start before
        # the whole transpose is done
        xTg = [pa.tile([P, KC, NT], F16, name=f"xTg{g}") for g in range(NSG)]
        wqh = pa.tile([P, KC, GC], F16, name="wqh")
        wkh = pa.tile([P, KC, GC], F16, name="wkh")
        wvh = pa.tile([P, KC, GC], F16, name="wvh")

        def qtkt_ns(w_sb, dst, bias_ap, mc, ns):
            ps = psum.tile([P, S], F32, tag="ps", name="ps_qk")[:, :NT]
            for kc in range(KC):
                nc.tensor.matmul(
                    ps,
                    lhsT=w_sb[:, kc, mc * P : (mc + 1) * P],
                    rhs=xTg[ns][:, kc, :],
                    start=(kc == 0),
                    stop=(kc == KC - 1),
                )
            out = dst[:, mc, ns * NT : (ns + 1) * NT]
            if bias_ap is not None:
                nc.vector.tensor_scalar_add(
                    out=out, in0=ps, scalar1=bias_ap[:, mc : mc + 1]
                )
            else:
                nc.vector.tensor_copy(out=out, in_=ps)

        # ---- fill: wq/wk via gpsimd queue, x split over sync+scalar queues,
        # wv trailing on sync; casts on DVE; xT copyouts split DVE/ScalarE;
        # QT/KT column group 0 interleaved as each x^T group completes.
        with tc.tile_pool(name="xin", bufs=1) as xin:
            ws_q = xin.tile([P, KC, GC], F32, name="ws_q", bufs=1)
            nc.gpsimd.dma_start(out=ws_q, in_=wq.rearrange("(kc p) m -> p kc m", p=P))
            nc.vector.tensor_copy(out=wqh, in_=ws_q)
            ws_k = xin.tile([P, KC, GC], F32, name="ws_k", bufs=1)
            nc.gpsimd.dma_start(out=ws_k, in_=wk.rearrange("(kc p) m -> p kc m", p=P))
            nc.vector.tensor_copy(out=wkh, in_=ws_k)
            xcs = []
            for sc in range(SC):
                xc = xin.tile([P, D], F32, tag="xc", name="xc", bufs=8)
                eng = nc.sync if sc % 2 == 0 else nc.scalar
                eng.dma_start(out=xc, in_=x[sc * P : (sc + 1) * P, :])
                xcs.append(xc)
            ws_v = xin.tile([P, KC, GC], F32, name="ws_v", bufs=1)
            nc.sync.dma_start(out=ws_v, in_=wv.rearrange("(kc p) m -> p kc m", p=P))
            nc.vector.tensor_copy(out=wvh, in_=ws_v)
            for sc in range(SC):
                ps = psum.tile([P, D], F32, tag="ps", name="ps_tr")
                for dc in range(KC):
                    nc.tensor.transpose(
                        ps[:, dc * P : (dc + 1) * P],
                        xcs[sc][:, dc * P : (dc + 1) * P],
                        idf,
                    )
                g, o = divmod(sc, 4)
                src = ps.rearrange("p (dc s) -> p dc s", dc=KC)
                dst_ap = xTg[g][:, :, o * P : (o + 1) * P]
                if sc % 2 == 0:
                    nc.vector.tensor_copy(out=dst_ap, in_=src)
                else:
                    nc.scalar.copy(out=dst_ap, in_=src)

        def v_chunk(sc):
            ps = psum.tile([P, S], F32, tag="ps", name="ps_v")[:, :GC]
            g, o = divmod(sc, 4)
            for kc in range(KC):
                nc.tensor.matmul(
                    ps,
                    lhsT=xTg[g][:, kc, o * P : (o + 1) * P],
                    rhs=wvh[:, kc, :],
                    start=(kc == 0),
                    stop=(kc == KC - 1),
                )
            nc.vector.tensor_copy(
                out=V[:, sc, :, 0:HD],
                in_=ps.rearrange("p (h d) -> p h d", h=HPC),
            )

        attnT = {}
        apool = ctx.enter_context(tc.tile_pool(name="attn", bufs=ATTNT_BUFS))
        accp = ctx.enter_context(tc.tile_pool(name="accp", bufs=ACC_BUFS))
        ACCT = {}

        def scores_exp(h, kc):
            mc = h // 2
            po = (h % 2) * HD
            ps = psum.tile([P, S], F32, tag="ps", name="ps_sc")
            for ns in range(NSG):
                nc.tensor.matmul(
                    ps[:, ns * NT : (ns + 1) * NT],
                    lhsT=KT[po : po + HD, mc, kc * P : (kc + 1) * P],
                    rhs=QT[po : po + HD, mc, ns * NT : (ns + 1) * NT],
                    start=True,
                    stop=True,
                )
            a = apool.tile([P, S], F16, tag="attnT", name="attnT")
            nc.scalar.activation(
                out=a, in_=ps, func=mybir.ActivationFunctionType.Exp, scale=0.125
            )
            attnT[(h, kc)] = a

        def attnv_pair(h, qcp, half):
            """attn@v for query chunks 2*qcp, 2*qcp+1 over one half of the
            key chunks; both psum regions live in one slot, copied out with
            a single strided DVE op."""
            ps = psum.tile([P, S], F32, tag="ps", name="ps_av")
            pvv = ps.rearrange("p (g r) -> p g r", g=NSG)[:, 0:2, 0 : HD + 1]
            for g in range(2):
                qc = 2 * qcp + g
                for i in range(SC // 2):
                    kcc = half * (SC // 2) + i
                    nc.tensor.matmul(
                        pvv[:, g],
                        lhsT=attnT[(h, kcc)][:, qc * P : (qc + 1) * P],
                        rhs=V[:, kcc, h, :],
                        start=(i == 0),
                        stop=(i == SC // 2 - 1),
                    )
            if half == 0:
                acc_t = accp.tile([P, 2, HD + 1], F32, tag="acc", name="acc")
                ACCT[(h, qcp)] = acc_t
                nc.vector.tensor_copy(out=acc_t, in_=pvv)
            else:
                acc_t = ACCT.pop((h, qcp))
                nc.vector.tensor_add(out=acc_t, in0=acc_t, in1=pvv)
                rec = small.tile([P, 2], F32, tag="rec", name="rec")
                nc.vector.reciprocal(out=rec, in_=acc_t[:, :, HD])
                for g in range(2):
                    nc.vector.tensor_scalar_mul(
                        out=AO[:, 2 * qcp + g, h * HD : (h + 1) * HD],
                        in0=acc_t[:, g, 0:HD],
                        scalar1=rec[:, g : g + 1],
                    )

        def aot_group(mc, sg):
            ps = psum.tile([P, NT], F16, tag="ps", name="ps_aot")
            for j in range(4):
                sc = sg * 4 + j
                nc.tensor.transpose(
                    ps[:, j * P : (j + 1) * P], AO[:, sc, mc * P : (mc + 1) * P], idh
                )
            nc.vector.tensor_copy(out=AOT[:, mc, sg * NT : (sg + 1) * NT], in_=ps)

        for ns in range(NSG):
            qtkt_ns(wqh, QT, bq_sb, 0, ns)
            qtkt_ns(wkh, KT, None, 0, ns)

        # ---- pipelined head loop
        for h in range(HPC):
            for kc in range(SC):
                if h == 0:
                    v_chunk(kc)
                # deferred qkv column groups for upcoming head pairs
                if h in (1, 2) and kc % 2 == 1:
                    mc = h  # h=1 -> mc1, h=2 -> mc2
                    i = kc // 2
                    if i < NSG:
                        qtkt_ns(wqh, QT, bq_sb, mc, i)
                    else:
                        qtkt_ns(wkh, KT, None, mc, i - NSG)
                # attn_out transposes for completed head pairs
                if h == 2 and 12 <= kc:
                    aot_group(0, kc - 12)
                if h == 4 and 12 <= kc:
                    aot_group(1, kc - 12)
                scores_exp(h, kc)
                if h >= 1 and kc < 8:
                    attnv_pair(h - 1, kc, 1)
                if kc >= 8:
                    attnv_pair(h, kc - 8, 0)

        # ---- tail: finish head 5, last transposes, projection
        with tc.tile_pool(name="tail", bufs=1) as tp, tc.tile_pool(
            name="yout", bufs=4
        ) as yp:
            wp32 = tp.tile([P, MC, D], F32, name="wp32")
            nc.gpsimd.dma_start(out=wp32, in_=wp.rearrange("(mc p) n -> p mc n", p=P))
            WPh = tp.tile([P, MC, D], F16, name="WPh")
            nc.vector.tensor_copy(out=WPh, in_=wp32)

            for qcp in range(SC // 2):
                attnv_pair(5, qcp, 1)
            for sg in range(NSG):
                aot_group(2, sg)

            for qc in range(SC):
                ps = psum.tile([P, S], F32, tag="ps", name="ps_pj")
                for mc in range(MC):
                    for n0 in range(0, D, NT):
                        n1 = min(n0 + NT, D)
                        nc.tensor.matmul(
                            ps[:, n0:n1],
                            lhsT=AOT[:, mc, qc * P : (qc + 1) * P],
                            rhs=WPh[:, mc, n0:n1],
                            start=(mc == 0),
                            stop=(mc == MC - 1),
                        )
                yt = yp.tile([P, D], F32, tag="yt", name="yt")
                nc.vector.tensor_copy(out=yt, in_=ps[:, :D])
                eng = nc.sync if qc % 2 == 0 else nc.scalar
                eng.dma_start(out=y[qc * P : (qc + 1) * P, :], in_=yt)


_CACHE = {}


def get_nc():
    if "nc" not in _CACHE:
        _CACHE["nc"] = _build_bass()
    return _CACHE["nc"]


LAST_RESULTS = None


def make_in_maps(inputs):
    x = np.asarray(inputs["x"], dtype=np.float32)
    w_qkv = np.asarray(inputs["w_qkv"], dtype=np.float32)
    b_qkv = np.asarray(inputs["b_qkv"], dtype=np.float32)
    w_proj = np.asarray(inputs["w_proj"], dtype=np.float32)
    in_maps = []
    for c in range(N_CORES):
        b, g = divmod(c, 2)
        g0 = GC * g
        in_maps.append(
            {
                "x": np.ascontiguousarray(x[b]),
                "wq": np.ascontiguousarray(w_qkv[:, g0 : g0 + GC]),
                "wk": np.ascontiguousarray(w_qkv[:, D + g0 : D + g0 + GC]),
                "wv": np.ascontiguousarray(w_qkv[:, 2 * D + g0 : 2 * D + g0 + GC]),
                "bq": np.ascontiguousarray(b_qkv[g0 : g0 + GC]),
                "wp": np.ascontiguousarray(w_proj[g0 : g0 + GC, :]),
            }
        )
    return in_maps


def kernel(x, w_qkv, b_qkv, w_proj, b_proj):
    global LAST_RESULTS
    from concourse.bass_utils import run_bass_kernel_spmd

    x = np.asarray(x, dtype=np.float32)
    w_qkv = np.asarray(w_qkv, dtype=np.float32)
    b_qkv = np.asarray(b_qkv, dtype=np.float32)
    w_proj = np.asarray(w_proj, dtype=np.float32)
    b_proj = np.asarray(b_proj, dtype=np.float32)

    nc = get_nc()
    in_maps = make_in_maps(
        {"x": x, "w_qkv": w_qkv, "b_qkv": b_qkv, "w_proj": w_proj}
    )

    res = run_bass_kernel_spmd(
        nc,
        in_maps,
        core_ids=list(range(N_CORES)),
        trace=bool(int(os.environ.get("MHA_TRACE", "0"))),
    )
    LAST_RESULTS = res

    b_eff = b_proj + b_qkv[2 * D : 3 * D] @ w_proj
    out = np.empty((B, S, D), dtype=np.float32)
    for b in range(B):
        out[b] = res.results[2 * b]["y"] + res.results[2 * b + 1]["y"] + b_eff
    return out


# revision 20
# speedup vs baseline: 1.0009x; 1.0009x over previous
"""Multi-head attention kernel for Trainium2, sharded over 8 NeuronCores.

Problem: B=4, S=2048, D=768, H=12 heads of dim 64.
  qkv = x @ w_qkv + b_qkv ; attention per head ; out = concat @ w_proj + b_proj

Sharding (batch x head-group): core c handles batch b = c//2 and head group
g = c%2 (6 heads, 384 qkv columns / w_proj rows).  Each core computes its
partial projection output; the host sums the two partials per batch and adds
the bias.

Algebraic simplifications (exact up to float rounding):
  - k-bias drops out of softmax entirely: (q+bq)@(k+bk)^T differs from
    (q+bq)@k^T by a per-query constant, which softmax cancels.
  - v-bias commutes with the normalized attention average, so it is folded
    into the host-side output bias: b_eff = b_proj + b_qkv[v] @ w_proj.

Device dataflow per core (fp16 matmul operands, fp32 PSUM accumulation):
  xT   = transpose(x)  (PE transpose, 4 column-group tiles)
  QT/KT = w^T @ x^T (+bq on Q)              [384, S] fp16
  V    = x @ wv, + ones column              [S, 6, 65] fp16
  per head h: scoresT = k_h @ q_h^T ; attnT = exp(scoresT/8)  (ScalarE)
              o|rowsum = attnT^T @ [v|1]  accumulated over key chunks
              attn_out = o * (1/rowsum)
  AOT  = transpose(attn_out) ; y = AOT^T @ w_proj_slice  [S, 768] fp32

The emission is software-pipelined around ScalarE (exp = 96 x [128,2048]
activations, the per-core bottleneck): each head iteration carries the
previous head's attn@v second half, this head's first half (two query
chunks merged into one PSUM slot + one strided copy), V chunks (head 0),
deferred QT/KT column groups, and AOT transposes for finished head pairs.
PSUM is two rotating [128,2048] fp32 slots; attn@v accumulates across the
two 8-key-chunk halves via small SBUF accumulators.  DMA load is spread
over the sync/scalar/gpsimd queues to keep the x stream on the critical
path.
"""

import os
import sys
from contextlib import ExitStack

import numpy as np

for _p in ("/opt/trn_rl_repo",):
    if os.path.isdir(_p) and _p not in sys.path:
        sys.path.insert(0, _p)

import concourse.bass as bass  # noqa: E402
import concourse.tile as tile  # noqa: E402
from concourse import bacc, mybir  # noqa: E402
from concourse.masks import make_identity  # noqa: E402

B, S, D, H = 4, 2048, 768, 12
HD = 64  # head dim
HPC = 6  # heads per core
GC = HPC * HD  # 384 qkv columns per core
P = 128
N_CORES = 8
SC = S // P  # 16 sequence chunks
KC = D // P  # 6 contraction chunks over D
MC = GC // P  # 3 column chunks per group
NT = 512  # matmul moving-dim tile
NSG = S // NT  # 4 sequence groups of 512

F32 = mybir.dt.float32
F16 = mybir.dt.float16

ATTNT_BUFS = 17  # [128, S] fp16 exp-output tiles in flight
ACC_BUFS = 10  # [128, 2, 65] fp32 attn@v pair accumulators in flight


def _build_bass():
    nc = bacc.Bacc("TRN2", target_bir_lowering=False, debug=False)
    x = nc.dram_tensor("x", (S, D), F32, kind="ExternalInput").ap()
    wq = nc.dram_tensor("wq", (D, GC), F32, kind="ExternalInput").ap()
    wk = nc.dram_tensor("wk", (D, GC), F32, kind="ExternalInput").ap()
    wv = nc.dram_tensor("wv", (D, GC), F32, kind="ExternalInput").ap()
    bq = nc.dram_tensor("bq", (GC,), F32, kind="ExternalInput").ap()
    wp = nc.dram_tensor("wp", (GC, D), F32, kind="ExternalInput").ap()
    y = nc.dram_tensor("y", (S, D), F32, kind="ExternalOutput").ap()
    with tile.TileContext(nc) as tc:
        _mha_kernel(tc, y, x, wq, wk, wv, bq, wp)
    nc.finalize()
    return nc


def _mha_kernel(tc, y, x, wq, wk, wv, bq, wp):
    nc = tc.nc
    with ExitStack() as ctx:
        # Two 4-bank [128, 2048] fp32 PSUM slots shared by every psum user.
        psum = ctx.enter_context(tc.tile_pool(name="psum", bufs=2, space="PSUM"))
        persist = ctx.enter_context(tc.tile_pool(name="persist", bufs=1))
        small = ctx.enter_context(tc.tile_pool(name="small", bufs=6))

        idf = persist.tile([P, P], F32, name="idf")
        make_identity(nc, idf)
        idh = persist.tile([P, P], F16, name="idh")
        make_identity(nc, idh)

        QT = persist.tile([P, MC, S], F16, name="QT")
        KT = persist.tile([P, MC, S], F16, name="KT")
        V = persist.tile([P, SC, HPC, HD + 1], F16, name="V")
        AO = persist.tile([P, SC, GC], F16, name="AO")
        AOT = persist.tile([P, MC, S], F16, name="AOT")
        bq_sb = persist.tile([P, MC], F32, name="bq_sb")
        nc.gpsimd.dma_start(out=bq_sb, in_=bq.rearrange("(mc p) -> p mc", p=P))
        nc.vector.memset(V[:, :, :, HD : HD + 1], 1.0)

        pa = ctx.enter_context(tc.tile_pool(name="pa", bufs=1))
        # x^T in four 512-query groups so the qkv matmuls can start before
        # the whole transpose is done
        xTg = [pa.tile([P, KC, NT], F16, name=f"xTg{g}") for g in range(NSG)]
        wqh = pa.tile([P, KC, GC], F16, name="wqh")
        wkh = pa.tile([P, KC, GC], F16, name="wkh")
        wvh = pa.tile([P, KC, GC], F16, name="wvh")

        def qtkt_ns(w_sb, dst, bias_ap, mc, ns):
            ps = psum.tile([P, S], F32, tag="ps", name="ps_qk")[:, :NT]
            for kc in range(KC):
                nc.tensor.matmul(
                    ps,
                    lhsT=w_sb[:, kc, mc * P : (mc + 1) * P],
                    rhs=xTg[ns][:, kc, :],
                    start=(kc == 0),
                    stop=(kc == KC - 1),
                )
            out = dst[:, mc, ns * NT : (ns + 1) * NT]
            if bias_ap is not None:
                nc.vector.tensor_scalar_add(
                    out=out, in0=ps, scalar1=bias_ap[:, mc : mc + 1]
                )
            else:
                nc.vector.tensor_copy(out=out, in_=ps)

        # ---- fill: wq/wk via gpsimd queue, x split over sync+scalar queues,
        # wv trailing on sync; casts on DVE; xT copyouts split DVE/ScalarE;
        # QT/KT column group 0 interleaved as each x^T group completes.
        with tc.tile_pool(name="xin", bufs=1) as xin:
            ws_q = xin.tile([P, KC, GC], F32, name="ws_q", bufs=1)
            nc.gpsimd.dma_start(out=ws_q, in_=wq.rearrange("(kc p) m -> p kc m", p=P))
            nc.vector.tensor_copy(out=wqh, in_=ws_q)
            ws_k = xin.tile([P, KC, GC], F32, name="ws_k", bufs=1)
            nc.gpsimd.dma_start(out=ws_k, in_=wk.rearrange("(kc p) m -> p kc m", p=P))
            nc.vector.tensor_copy(out=wkh, in_=ws_k)
            xcs = []
            for sc in range(SC):
                xc = xin.tile([P, D], F32, tag="xc", name="xc", bufs=8)
                eng = nc.sync if sc % 2 == 0 else nc.scalar
                eng.dma_start(out=xc, in_=x[sc * P : (sc + 1) * P, :])
                xcs.append(xc)
            ws_v = xin.tile([P, KC, GC], F32, name="ws_v", bufs=1)
            nc.sync.dma_start(out=ws_v, in_=wv.rearrange("(kc p) m -> p kc m", p=P))
            nc.vector.tensor_copy(out=wvh, in_=ws_v)
            for sc in range(SC):
                ps = psum.tile([P, D], F32, tag="ps", name="ps_tr")
                for dc in range(KC):
                    nc.tensor.transpose(
                        ps[:, dc * P : (dc + 1) * P],
                        xcs[sc][:, dc * P : (dc + 1) * P],
                        idf,
                    )
                g, o = divmod(sc, 4)
                src = ps.rearrange("p (dc s) -> p dc s", dc=KC)
                dst_ap = xTg[g][:, :, o * P : (o + 1) * P]
                if sc % 2 == 0:
                    nc.vector.tensor_copy(out=dst_ap, in_=src)
                else:
                    nc.scalar.copy(out=dst_ap, in_=src)

        def v_chunk(sc):
            ps = psum.tile([P, S], F32, tag="ps", name="ps_v")[:, :GC]
            g, o = divmod(sc, 4)
            for kc in range(KC):
                nc.tensor.matmul(
                    ps,
                    lhsT=xTg[g][:, kc, o * P : (o + 1) * P],
                    rhs=wvh[:, kc, :],
                    start=(kc == 0),
                    stop=(kc == KC - 1),
                )
            nc.vector.tensor_copy(
                out=V[:, sc, :, 0:HD],
                in_=ps.rearrange("p (h d) -> p h d", h=HPC),
            )

        attnT = {}
        apool = ctx.enter_context(tc.tile_pool(name="attn", bufs=ATTNT_BUFS))
        accp = ctx.enter_context(tc.tile_pool(name="accp", bufs=ACC_BUFS))
        ACCT = {}

        def scores_exp(h, kc):
            mc = h // 2
            po = (h % 2) * HD
            ps = psum.tile([P, S], F32, tag="ps", name="ps_sc")
            for ns in range(NSG):
                nc.tensor.matmul(
                    ps[:, ns * NT : (ns + 1) * NT],
                    lhsT=KT[po : po + HD, mc, kc * P : (kc + 1) * P],
                    rhs=QT[po : po + HD, mc, ns * NT : (ns + 1) * NT],
                    start=True,
                    stop=True,
                )
            a = apool.tile([P, S], F16, tag="attnT", name="attnT")
            nc.scalar.activation(
                out=a, in_=ps, func=mybir.ActivationFunctionType.Exp, scale=0.125
            )
            attnT[(h, kc)] = a

        def attnv_pair(h, qcp, half):
            """attn@v for query chunks 2*qcp, 2*qcp+1 over one half of the
            key chunks; both psum regions live in one slot, copied out with
            a single strided DVE op."""
            ps = psum.tile([P, S], F32, tag="ps", name="ps_av")
            pvv = ps.rearrange("p (g r) -> p g r", g=NSG)[:, 0:2, 0 : HD + 1]
            for g in range(2):
                qc = 2 * qcp + g
                for i in range(SC // 2):
                    kcc = half * (SC // 2) + i
                    nc.tensor.matmul(
                        pvv[:, g],
                        lhsT=attnT[(h, kcc)][:, qc * P : (qc + 1) * P],
                        rhs=V[:, kcc, h, :],
                        start=(i == 0),
                        stop=(i == SC // 2 - 1),
                    )
            if half == 0:
                acc_t = accp.tile([P, 2, HD + 1], F32, tag="acc", name="acc")
                ACCT[(h, qcp)] = acc_t
                nc.vector.tensor_copy(out=acc_t, in_=pvv)
            else:
                acc_t = ACCT.pop((h, qcp))
                nc.vector.tensor_add(out=acc_t, in0=acc_t, in1=pvv)
                rec = small.tile([P, 2], F32, tag="rec", name="rec")
                nc.vector.reciprocal(out=rec, in_=acc_t[:, :, HD])
                for g in range(2):
                    nc.vector.tensor_scalar_mul(
                        out=AO[:, 2 * qcp + g, h * HD : (h + 1) * HD],
                        in0=acc_t[:, g, 0:HD],
                        scalar1=rec[:, g : g + 1],
                    )

        def aot_group(mc, sg):
            ps = psum.tile([P, NT], F16, tag="ps", name="ps_aot")
            for j in range(4):
                sc = sg * 4 + j
                nc.tensor.transpose(
                    ps[:, j * P : (j + 1) * P], AO[:, sc, mc * P : (mc + 1) * P], idh
                )
            nc.vector.tensor_copy(out=AOT[:, mc, sg * NT : (sg + 1) * NT], in_=ps)

        for ns in range(NSG):
            qtkt_ns(wqh, QT, bq_sb, 0, ns)
            qtkt_ns(wkh, KT, None, 0, ns)

        # ---- pipelined head loop
        for h in range(HPC):
            for kc in range(SC):
                if h == 0:
                    v_chunk(kc)
                # deferred qkv column groups for upcoming head pairs
                if h in (1, 2) and kc % 2 == 1:
                    mc = h  # h=1 -> mc1, h=2 -> mc2
                    i = kc // 2
                    if i < NSG:
                        qtkt_ns(wqh, QT, bq_sb, mc, i)
                    else:
                        qtkt_ns(wkh, KT, None, mc, i - NSG)
                # attn_out transposes for completed head pairs
                if h == 2 and 12 <= kc:
                    aot_group(0, kc - 12)
                if h == 4 and 12 <= kc:
                    aot_group(1, kc - 12)
                scores_exp(h, kc)
                if h >= 1 and kc < 8:
                    attnv_pair(h - 1, kc, 1)
                if kc >= 8:
                    attnv_pair(h, kc - 8, 0)

        # ---- tail: finish head 5, last transposes, projection
        with tc.tile_pool(name="tail", bufs=1) as tp, tc.tile_pool(
            name="yout", bufs=4
        ) as yp:
            wp32 = tp.tile([P, MC, D], F32, name="wp32")
            nc.gpsimd.dma_start(out=wp32, in_=wp.rearrange("(mc p) n -> p mc n", p=P))
            WPh = tp.tile([P, MC, D], F16, name="WPh")
            nc.vector.tensor_copy(out=WPh, in_=wp32)

            for qcp in range(SC // 2):
                attnv_pair(5, qcp, 1)
            for sg in range(NSG):
                aot_group(2, sg)

            for qc in range(SC):
                ps = psum.tile([P, S], F32, tag="ps", name="ps_pj")
                for mc in range(MC):
                    for n0 in range(0, D, NT):
                        n1 = min(n0 + NT, D)
                        nc.tensor.matmul(
                            ps[:, n0:n1],
                            lhsT=AOT[:, mc, qc * P : (qc + 1) * P],
                            rhs=WPh[:, mc, n0:n1],
                            start=(mc == 0),
                            stop=(mc == MC - 1),
                        )
                yt = yp.tile([P, D], F32, tag="yt", name="yt")
                if qc % 2 == 0:
                    nc.vector.tensor_copy(out=yt, in_=ps[:, :D])
                else:
                    nc.scalar.copy(out=yt, in_=ps[:, :D])
                eng = nc.sync if qc % 2 == 0 else nc.scalar
                eng.dma_start(out=y[qc * P : (qc + 1) * P, :], in_=yt)


_CACHE = {}


def get_nc():
    if "nc" not in _CACHE:
        _CACHE["nc"] = _build_bass()
    return _CACHE["nc"]


LAST_RESULTS = None


def make_in_maps(inputs):
    x = np.asarray(inputs["x"], dtype=np.float32)
    w_qkv = np.asarray(inputs["w_qkv"], dtype=np.float32)
    b_qkv = np.asarray(inputs["b_qkv"], dtype=np.float32)
    w_proj = np.asarray(inputs["w_proj"], dtype=np.float32)
    in_maps = []
    for c in range(N_CORES):
        b, g = divmod(c, 2)
        g0 = GC * g
        in_maps.append(
            {
                "x": np.ascontiguousarray(x[b]),
                "wq": np.ascontiguousarray(w_qkv[:, g0 : g0 + GC]),
                "wk": np.ascontiguousarray(w_qkv[:, D + g0 : D + g0 + GC]),
                "wv": np.ascontiguousarray(w_qkv[:, 2 * D + g0 : 2 * D + g0 + GC]),
                "bq": np.ascontiguousarray(b_qkv[g0 : g0 + GC]),
                "wp": np.ascontiguousarray(w_proj[g0 : g0 + GC, :]),
            }
        )
    return in_maps


def kernel(x, w_qkv, b_qkv, w_proj, b_proj):
    global LAST_RESULTS
    from concourse.bass_utils import run_bass_kernel_spmd

    x = np.asarray(x, dtype=np.float32)
    w_qkv = np.asarray(w_qkv, dtype=np.float32)
    b_qkv = np.asarray(b_qkv, dtype=np.float32)
    w_proj = np.asarray(w_proj, dtype=np.float32)
    b_proj = np.asarray(b_proj, dtype=np.float32)

    nc = get_nc()
    in_maps = make_in_maps(
        {"x": x, "w_qkv": w_qkv, "b_qkv": b_qkv, "w_proj": w_proj}
    )

    res = run_bass_kernel_spmd(
        nc,
        in_maps,
        core_ids=list(range(N_CORES)),
        trace=bool(int(os.environ.get("MHA_TRACE", "0"))),
    )
    LAST_RESULTS = res

    b_eff = b_proj + b_qkv[2 * D : 3 * D] @ w_proj
    out = np.empty((B, S, D), dtype=np.float32)
    for b in range(B):
        out[b] = res.results[2 * b]["y"] + res.results[2 * b + 1]["y"] + b_eff
    return out
